# revision 33
# baseline (speedup 1.0000x reference)
"""Trainium2 Bass kernel for nn_NodeAttentionPerMetaPath (GAT-style node attention).

Reference computation (N=8192, F_IN=256, d=64):
    h      = x @ trans                      # [N, d]
    e1     = h @ attn[:d];  e2 = h @ attn[d:]
    scores = leaky_relu(e1 + e2.T, 0.2)     # [N, N]
    masked = where(mask==0, -1e15, scores)
    out    = softmax(masked, axis=1) @ h    # [N, d]

Sharding: rows r across 8 cores (1024 each); every core computes the full
h/e locally from a replicated fp16 x (no collectives at all).

Algebra (exp monotone, alpha<1):
    exp(leaky(e1+e2)) / exp(a*e1) = B2[j] * max(C[r]*D[j], 1)
    C = exp(.8 e1), D = exp(.8 e2), B2 = exp(.2 e2), C*D*B2 = C*exp(e2)
    out = (P @ h) / (P @ 1),  P = mask * B2 * max(CD, 1)
        = mask * max(C_rep * db, b2)        (one fused 4x tensor_scalar)

Layout: scores live TRANSPOSED [j-part, r-free] from the start.  The mask
is packed to fp16 on the host and transpose-loaded straight from DRAM by
the DMA XBAR (dma_start(transpose=True)), so the [N,N] work needs NO PE
transposes and NO PSUM->SBUF copies: per 128-j chunk it is one TS, one
masked multiply (split DVE/GPSIMD), and one accumulated matmul whose
ones-column yields the softmax denominator for free.
"""

from contextlib import ExitStack

import numpy as np

import concourse.bass as bass
import concourse.bacc as bacc
import concourse.mybir as mybir
import concourse.tile as tile
from concourse.bass_utils import run_bass_kernel_spmd
from concourse.masks import make_identity

f32 = mybir.dt.float32
fp16 = mybir.dt.float16

Exp = mybir.ActivationFunctionType.Exp

N_CORES = 8
N = 8192
F_IN = 256
D = 64
ALPHA = 0.2

R = N // N_CORES   # rows per core
JC = N // 128      # j-chunks
NG = 16            # mask stream groups
CG = JC // NG      # chunks per group (4)
HQ = 16            # h-compute groups (4 chunks each)

# masked-multiply split per chunk-PAIR [128, 2048]:
# columns [0:TT_SPLIT] on DVE, rest on GPSIMD
TT_SPLIT = 1216


def build_kernel(ctx: ExitStack, tc: tile.TileContext, mask16_c, x_h, trans_h, transT_h, a12h, outT):
    nc = tc.nc

    singles = ctx.enter_context(tc.tile_pool(name="singles", bufs=1))
    hps_pool = ctx.enter_context(tc.tile_pool(name="hps", bufs=2, space="PSUM"))
    ps_o = ctx.enter_context(tc.tile_pool(name="ps_o", bufs=1, space="PSUM"))
    ps_r = ctx.enter_context(tc.tile_pool(name="ps_r", bufs=1, space="PSUM"))
    work = ctx.enter_context(tc.tile_pool(name="work", bufs=3))
    outp = ctx.enter_context(tc.tile_pool(name="outp", bufs=1))
    dram = ctx.enter_context(tc.tile_pool(name="dram", bufs=1, space="DRAM"))

    # ---- DMA streams.  mask and x arrive HOST-TRANSPOSED, so everything
    # is a plain contiguous-line DMA load; the mask stream is split across
    # BOTH HWDGE queues (sync takes even groups, scalar odd) for full
    # bandwidth with no XBAR serialization.
    trans_w = singles.tile([128, 2, D + 2], fp16)
    nc.scalar.dma_start(
        out=trans_w[:, :, 0:D], in_=trans_h.rearrange("(c p) d -> p c d", p=128)
    )
    transT_sb = singles.tile([D, F_IN], fp16)
    nc.scalar.dma_start(out=transT_sb, in_=transT_h[:, :])
    a12_sb = singles.tile([D, 2], fp16)
    nc.scalar.dma_start(out=a12_sb, in_=a12h[:, :])
    # xT[p, fc, j] = xT_h[fc*128+p, j] (host-transposed x), in three
    # pieces (j 0:1024 / 1024:4096 / 4096:8192).  The tiny first piece
    # lands in a few us so own-row h (and C_rep, which gates the main
    # loop) start almost immediately; the rest interleaves with the mask
    # stream across both HWDGE queues.
    XSL = [(0, 1024), (1024, 4096), (4096, N)]
    xTq = [
        singles.tile([128, 2, b - a], fp16, tag=f"xTq{i}", name=f"xTq{i}")
        for i, (a, b) in enumerate(XSL)
    ]

    def load_x(i, eng):
        a, b = XSL[i]
        eng.dma_start(out=xTq[i], in_=x_h[:, a:b].rearrange("(c p) j -> p c j", p=128))

    mtp = ctx.enter_context(tc.tile_pool(name="mtp", bufs=8))
    mts = [None] * NG

    def load_mask(g, eng):
        # mt_g[p, k, r] = maskT[(g*CG+k)*128+p, r]; rotating buffer pool:
        # the stream self-throttles on consumption
        mt = mtp.tile([128, CG, R], fp16, tag="mt", name=f"mt{g}")
        rows = slice(g * CG * 128, (g + 1) * CG * 128)
        eng.dma_start(out=mt, in_=mask16_c[rows, :].rearrange("(k p) r -> p k r", p=128))
        mts[g] = mt

    # sync queue: x0, g0, x1, then even groups; scalar: g1, g3, x2, then odds
    load_x(0, nc.sync)
    load_mask(0, nc.sync)
    load_x(1, nc.sync)
    for g in range(2, NG, 2):
        load_mask(g, nc.sync)
    load_mask(1, nc.scalar)
    load_mask(3, nc.scalar)
    load_x(2, nc.scalar)
    for g in range(5, NG, 2):
        load_mask(g, nc.scalar)

    ident = singles.tile([128, 128], fp16)
    make_identity(nc, ident)
    ones_row_f = singles.tile([1, D], f32)
    nc.vector.memset(ones_row_f, 1.0)

    # ---- w12 = trans.T @ a12  -> moving-operand columns 64:66
    for fc in range(2):
        wps = hps_pool.tile([128, 2], f32, tag="wps", bufs=1)
        nc.tensor.matmul(
            wps, transT_sb[:, fc * 128 : (fc + 1) * 128], a12_sb, start=True, stop=True
        )
        nc.vector.tensor_copy(trans_w[:, fc, D : D + 2], wps)

    # ---- full h/e per 4-chunk groups.  Inputs are rolled per core so the
    # own 1024 rows are always chunks 0..7 (groups 0 and 1).
    h_sb = singles.tile([128, JC, D + 1], fp16)   # [j%128, jc, h | 1]
    nc.vector.memset(h_sb[:, :, D], 1.0)
    c_all = singles.tile([128, JC], fp16)   # exp(.8 e1)
    db_all = singles.tile([128, JC], f32)  # exp(e2)
    b2_all = singles.tile([128, JC], f32)  # exp(.2 e2)
    C_rep = singles.tile([128, R], fp16)

    def h_group(q):
        hps = hps_pool.tile([128, 4, D + 2], f32, tag="hps", name=f"hps{q}")
        for k in range(4):
            jc = q * 4 + k
            xi = 0 if jc < 8 else (1 if jc < 32 else 2)
            j0 = (jc - (0, 8, 32)[xi]) * 128
            for fc in range(2):
                nc.tensor.matmul(
                    hps[:, k, :],
                    xTq[xi][:, fc, j0 : j0 + 128],
                    trans_w[:, fc, :],
                    start=(fc == 0),
                    stop=(fc == 1),
                )
        cols = slice(q * 4, q * 4 + 4)
        nc.scalar.copy(h_sb[:, cols, 0:D], hps[:, :, 0:D])
        nc.scalar.activation(c_all[:, cols], hps[:, :, D], Exp, scale=1.0 - ALPHA)
        nc.scalar.activation(db_all[:, cols], hps[:, :, D + 1], Exp, scale=1.0)
        nc.scalar.activation(b2_all[:, cols], hps[:, :, D + 1], Exp, scale=ALPHA)

    # own-row groups first: they feed C_rep, which gates the main loop.
    # Groups 2..15 are emitted inside the main loop, two pairs ahead of use.
    for q in range(2):
        h_group(q)
        if q == 1:
            # ---- C_rep: own-row C values replicated across partitions.
            # Transpose each own column of c_all to a [1, 128] row (PE),
            # assemble [1, R], then broadcast via K=1 matmuls.
            # (No DRAM roundtrip, no DMA in the middle of the XBAR stream.)
            c_row1 = singles.tile([1, R], fp16)
            crps = hps_pool.tile([1, 8, 128], fp16, tag="crps", bufs=1)
            for rb in range(8):
                nc.tensor.transpose(crps[:, rb, :], c_all[:, rb : rb + 1], ident)
            nc.vector.tensor_copy(c_row1, crps.rearrange("p a b -> p (a b)"))
            ones_col = singles.tile([1, 128], fp16)
            nc.vector.memset(ones_col, 1.0)
            for half in range(2):
                crp = hps_pool.tile([128, 512], f32, tag="crp", bufs=1)
                nc.tensor.matmul(
                    crp,
                    ones_col,
                    c_row1[:, half * 512 : (half + 1) * 512],
                    start=True,
                    stop=True,
                )
                nc.vector.tensor_copy(C_rep[:, half * 512 : (half + 1) * 512], crp)

    # ---- main loop: chunk-PAIRS.  Two 4x TS fills, one DVE TT + one
    # GPSIMD TT over the flattened [128, 2048], four accumulated matmuls.
    po = [ps_o.tile([D + 1, 512], f32, tag=f"po{i}", name=f"po{i}") for i in range(2)]
    M = mybir.AluOpType.mult
    # Per pair: one vp fill (two TS), then ONE DVE TT and ONE GPSIMD TT
    # into two single-producer q tiles.  The DVE/GPSIMD column boundary
    # alternates 1536/512 and 1024/1024 to balance engine load while every
    # matmul still reads a 512-slice with a single producer.
    NP = JC // 2
    for p in range(NP):
        jc0 = 2 * p
        mt = mts[jc0 // CG]
        k = jc0 % CG
        vp = work.tile([128, 2, R], fp16, tag="vp", bufs=8, name=f"vp{p}")
        for half in range(2):
            jc = jc0 + half
            # v = max(C_rep * exp(e2[jc]), exp(.2 e2[jc]))  (= B2 * max(CD, 1))
            nc.vector.tensor_scalar(
                vp[:, half, :],
                C_rep,
                db_all[:, jc : jc + 1],
                b2_all[:, jc : jc + 1],
                M,
                mybir.AluOpType.max,
            )
        bnd = 1536 if p % 2 == 0 else 1024
        vf = vp.rearrange("p a b -> p (a b)")
        mf = mt[:, k : k + 2, :].rearrange("p a b -> p (a b)")
        qD = work.tile([128, 1536], fp16, tag="qD", bufs=8, name=f"qD{p}")
        qG = work.tile([128, 1024], fp16, tag="qG", bufs=8, name=f"qG{p}")
        nc.gpsimd.tensor_tensor(qG[:, 0 : 2048 - bnd], vf[:, bnd:], mf[:, bnd:], M)
        nc.vector.tensor_tensor(qD[:, 0:bnd], vf[:, 0:bnd], mf[:, 0:bnd], M)
        if p + 2 < HQ:
            h_group(p + 2)
        for half in range(2):
            jc = jc0 + half
            for i in range(2):
                c0 = half * 1024 + i * 512
                src_q = qD[:, c0 : c0 + 512] if c0 < bnd else qG[:, c0 - bnd : c0 - bnd + 512]
                nc.tensor.matmul(
                    po[i],
                    h_sb[:, jc, :],
                    src_q,
                    start=(jc == 0),
                    stop=(jc == JC - 1),
                )

    # ---- normalize: out = numer * (1/denom).  Broadcast the denominator
    # row via a K=1 outer product FIRST, then reciprocal on 64 lanes.
    # The two halves are interleaved to pipeline across engines.
    dcps, rrs, rsbs, ots = [], [], [], []
    for i in range(2):
        dcp = outp.tile([1, 512], f32, tag=f"dcp{i}", name=f"dcp{i}")
        nc.vector.tensor_copy(dcp, po[i][D : D + 1, :])
        dcps.append(dcp)
    for i in range(2):
        rr = ps_r.tile([D, 512], f32, tag="rr", name=f"rr{i}")
        nc.tensor.matmul(rr, ones_row_f, dcps[i], start=True, stop=True)
        rr_sb = outp.tile([D, 512], f32, tag=f"rr_sb{i}", name=f"rr_sb{i}")
        nc.vector.reciprocal(rr_sb, rr)
        rsbs.append(rr_sb)
    for i in range(2):
        o_t = outp.tile([D, 512], f32, tag=f"o_t{i}", name=f"o_t{i}")
        nc.vector.tensor_tensor(o_t, po[i][0:D, :], rsbs[i], M)
        nc.gpsimd.dma_start(out=outT[:, i * 512 : (i + 1) * 512], in_=o_t)


def build_nc():
    nc = bacc.Bacc("TRN2", num_devices=N_CORES)
    mask16_c = nc.dram_tensor("mask16_c", [N, R], fp16, kind="ExternalInput")
    x_h = nc.dram_tensor("x_h", [F_IN, N], fp16, kind="ExternalInput")
    trans_h = nc.dram_tensor("trans_h", [F_IN, D], fp16, kind="ExternalInput")
    transT_h = nc.dram_tensor("transT_h", [D, F_IN], fp16, kind="ExternalInput")
    a12h = nc.dram_tensor("a12h", [D, 2], fp16, kind="ExternalInput")
    outT = nc.dram_tensor("outT", [D, R], f32, kind="ExternalOutput")
    with ExitStack() as ctx:
        tc = ctx.enter_context(tile.TileContext(nc))
        build_kernel(
            ctx, tc, mask16_c[:, :], x_h[:, :], trans_h[:, :],
            transT_h[:, :], a12h[:, :], outT[:, :],
        )
    nc.compile()
    return nc


LAST_RESULTS = None


def kernel(x, mask, trans, attn, _trace=False):
    x16 = np.ascontiguousarray(np.asarray(x), dtype=np.float16)
    mask16 = np.ascontiguousarray(np.asarray(mask), dtype=np.float16)
    trans16 = np.ascontiguousarray(np.asarray(trans), dtype=np.float16)
    transT16 = np.ascontiguousarray(np.asarray(trans).T, dtype=np.float16)
    attn = np.asarray(attn, dtype=np.float16)
    a12 = np.ascontiguousarray(np.concatenate([attn[:D], attn[D:]], axis=1))

    nc = build_nc()
    # identical SPMD program on every core: roll x rows / mask columns by
    # -c*R so each core's own rows are always j-chunks 0..7 (a column
    # permutation inside the softmax sum; the result is unchanged)
    in_maps = [
        {
            "mask16_c": np.ascontiguousarray(
                np.roll(mask16[c * R : (c + 1) * R], -c * R, axis=1).T
            ),
            "x_h": np.ascontiguousarray(np.roll(x16, -c * R, axis=0).T),
            "trans_h": trans16,
            "transT_h": transT16,
            "a12h": a12,
        }
        for c in range(N_CORES)
    ]
    res = run_bass_kernel_spmd(nc, in_maps, list(range(N_CORES)), trace=_trace)
    global LAST_RESULTS
    LAST_RESULTS = res
    out = np.concatenate(
        [res.results[c]["outT"].T for c in range(N_CORES)], axis=0
    )
    return np.ascontiguousarray(out, dtype=np.float32)


if __name__ == "__main__":
    nc = build_nc()
    print("built OK")


# revision 34
# speedup vs baseline: 1.1622x; 1.1622x over previous
"""Trainium2 Bass kernel for nn_NodeAttentionPerMetaPath (GAT-style node attention).

Reference computation (N=8192, F_IN=256, d=64):
    h      = x @ trans                      # [N, d]
    e1     = h @ attn[:d];  e2 = h @ attn[d:]
    scores = leaky_relu(e1 + e2.T, 0.2)     # [N, N]
    masked = where(mask==0, -1e15, scores)
    out    = softmax(masked, axis=1) @ h    # [N, d]

Sharding: rows r across 8 cores (1024 each); every core computes the full
h/e locally from a replicated fp16 x (no collectives at all).

Algebra (exp monotone, alpha<1):
    exp(leaky(e1+e2)) / exp(a*e1) = B2[j] * max(C[r]*D[j], 1)
    C = exp(.8 e1), D = exp(.8 e2), B2 = exp(.2 e2), C*D*B2 = C*exp(e2)
    out = (P @ h) / (P @ 1),  P = mask * B2 * max(CD, 1)
        = mask * max(C_rep * db, b2)        (one fused 4x tensor_scalar)

Layout: scores live TRANSPOSED [j-part, r-free] from the start.  The mask
is packed to fp16 on the host and transpose-loaded straight from DRAM by
the DMA XBAR (dma_start(transpose=True)), so the [N,N] work needs NO PE
transposes and NO PSUM->SBUF copies: per 128-j chunk it is one TS, one
masked multiply (split DVE/GPSIMD), and one accumulated matmul whose
ones-column yields the softmax denominator for free.
"""

from contextlib import ExitStack

import numpy as np

import concourse.bass as bass
import concourse.bacc as bacc
import concourse.mybir as mybir
import concourse.tile as tile
from concourse.bass_utils import run_bass_kernel_spmd
from concourse.masks import make_identity

f32 = mybir.dt.float32
fp16 = mybir.dt.float16

Exp = mybir.ActivationFunctionType.Exp

N_CORES = 8
N = 8192
F_IN = 256
D = 64
ALPHA = 0.2

R = N // N_CORES   # rows per core
JC = N // 128      # j-chunks
NG = 16            # mask stream groups
CG = JC // NG      # chunks per group (4)
HQ = 16            # h-compute groups (4 chunks each)

# masked-multiply split per chunk-PAIR [128, 2048]:
# columns [0:TT_SPLIT] on DVE, rest on GPSIMD
TT_SPLIT = 1216


def build_kernel(ctx: ExitStack, tc: tile.TileContext, mask16_c, x_h, trans_h, transT_h, a12h, outT):
    nc = tc.nc

    singles = ctx.enter_context(tc.tile_pool(name="singles", bufs=1))
    hps_pool = ctx.enter_context(tc.tile_pool(name="hps", bufs=2, space="PSUM"))
    ps_o = ctx.enter_context(tc.tile_pool(name="ps_o", bufs=1, space="PSUM"))
    ps_r = ctx.enter_context(tc.tile_pool(name="ps_r", bufs=1, space="PSUM"))
    work = ctx.enter_context(tc.tile_pool(name="work", bufs=3))
    outp = ctx.enter_context(tc.tile_pool(name="outp", bufs=1))
    dram = ctx.enter_context(tc.tile_pool(name="dram", bufs=1, space="DRAM"))

    # ---- DMA streams.  mask and x arrive HOST-TRANSPOSED, so everything
    # is a plain contiguous-line DMA load; the mask stream is split across
    # BOTH HWDGE queues (sync takes even groups, scalar odd) for full
    # bandwidth with no XBAR serialization.
    trans_w = singles.tile([128, 2, D + 2], fp16)
    nc.scalar.dma_start(
        out=trans_w[:, :, 0:D], in_=trans_h.rearrange("(c p) d -> p c d", p=128)
    )
    transT_sb = singles.tile([D, F_IN], fp16)
    nc.scalar.dma_start(out=transT_sb, in_=transT_h[:, :])
    a12_sb = singles.tile([D, 2], fp16)
    nc.scalar.dma_start(out=a12_sb, in_=a12h[:, :])
    # xT[p, fc, j] = xT_h[fc*128+p, j] (host-transposed x), in three
    # pieces (j 0:1024 / 1024:4096 / 4096:8192).  The tiny first piece
    # lands in a few us so own-row h (and C_rep, which gates the main
    # loop) start almost immediately; the rest interleaves with the mask
    # stream across both HWDGE queues.
    XSL = [(0, 1024), (1024, 4096), (4096, N)]
    xTq = [
        singles.tile([128, 2, b - a], fp16, tag=f"xTq{i}", name=f"xTq{i}")
        for i, (a, b) in enumerate(XSL)
    ]

    def load_x(i, eng):
        a, b = XSL[i]
        eng.dma_start(out=xTq[i], in_=x_h[:, a:b].rearrange("(c p) j -> p c j", p=128))

    mtp = ctx.enter_context(tc.tile_pool(name="mtp", bufs=8))
    mts = [None] * NG

    def load_mask(g, eng):
        # mt_g[p, k, r] = maskT[(g*CG+k)*128+p, r]; rotating buffer pool:
        # the stream self-throttles on consumption
        mt = mtp.tile([128, CG, R], fp16, tag="mt", name=f"mt{g}")
        rows = slice(g * CG * 128, (g + 1) * CG * 128)
        eng.dma_start(out=mt, in_=mask16_c[rows, :].rearrange("(k p) r -> p k r", p=128))
        mts[g] = mt

    # sync queue: x0, g0, x1, then even groups; scalar: g1, g3, x2, then odds
    load_x(0, nc.sync)
    load_mask(0, nc.sync)
    load_x(1, nc.sync)
    for g in range(2, NG, 2):
        load_mask(g, nc.sync)
    load_mask(1, nc.scalar)
    load_mask(3, nc.scalar)
    load_x(2, nc.scalar)
    for g in range(5, NG, 2):
        load_mask(g, nc.scalar)

    ident = singles.tile([128, 128], fp16)
    make_identity(nc, ident)
    ones_row_f = singles.tile([1, D], f32)
    nc.vector.memset(ones_row_f, 1.0)

    # ---- w12 = trans.T @ a12  -> moving-operand columns 64:66
    for fc in range(2):
        wps = hps_pool.tile([128, 2], f32, tag="wps", bufs=1)
        nc.tensor.matmul(
            wps, transT_sb[:, fc * 128 : (fc + 1) * 128], a12_sb, start=True, stop=True
        )
        nc.vector.tensor_copy(trans_w[:, fc, D : D + 2], wps)

    # ---- full h/e per 4-chunk groups.  Inputs are rolled per core so the
    # own 1024 rows are always chunks 0..7 (groups 0 and 1).
    h_sb = singles.tile([128, JC, D + 1], fp16)   # [j%128, jc, h | 1]
    nc.vector.memset(h_sb[:, :, D], 1.0)
    c_all = singles.tile([128, JC], fp16)   # exp(.8 e1)
    db_all = singles.tile([128, JC], f32)  # exp(e2)
    b2_all = singles.tile([128, JC], f32)  # exp(.2 e2)
    C_rep = singles.tile([128, R], fp16)

    def h_group(q):
        hps = hps_pool.tile([128, 4, D + 2], f32, tag="hps", name=f"hps{q}")
        for k in range(4):
            jc = q * 4 + k
            xi = 0 if jc < 8 else (1 if jc < 32 else 2)
            j0 = (jc - (0, 8, 32)[xi]) * 128
            for fc in range(2):
                nc.tensor.matmul(
                    hps[:, k, :],
                    xTq[xi][:, fc, j0 : j0 + 128],
                    trans_w[:, fc, :],
                    start=(fc == 0),
                    stop=(fc == 1),
                )
        cols = slice(q * 4, q * 4 + 4)
        nc.scalar.copy(h_sb[:, cols, 0:D], hps[:, :, 0:D])
        nc.scalar.activation(c_all[:, cols], hps[:, :, D], Exp, scale=1.0 - ALPHA)
        nc.scalar.activation(db_all[:, cols], hps[:, :, D + 1], Exp, scale=1.0)
        nc.scalar.activation(b2_all[:, cols], hps[:, :, D + 1], Exp, scale=ALPHA)

    # own-row groups first: they feed C_rep, which gates the main loop.
    for q in range(HQ):
        h_group(q)
        if q == 1:
            # ---- C_rep: own-row C values replicated across partitions.
            # Transpose each own column of c_all to a [1, 128] row (PE),
            # assemble [1, R], then broadcast via K=1 matmuls.
            # (No DRAM roundtrip, no DMA in the middle of the XBAR stream.)
            c_row1 = singles.tile([1, R], fp16)
            crps = hps_pool.tile([1, 8, 128], fp16, tag="crps", bufs=1)
            for rb in range(8):
                nc.tensor.transpose(crps[:, rb, :], c_all[:, rb : rb + 1], ident)
            nc.vector.tensor_copy(c_row1, crps.rearrange("p a b -> p (a b)"))
            ones_col = singles.tile([1, 128], fp16)
            nc.vector.memset(ones_col, 1.0)
            for half in range(2):
                crp = hps_pool.tile([128, 512], f32, tag="crp", bufs=1)
                nc.tensor.matmul(
                    crp,
                    ones_col,
                    c_row1[:, half * 512 : (half + 1) * 512],
                    start=True,
                    stop=True,
                )
                nc.vector.tensor_copy(C_rep[:, half * 512 : (half + 1) * 512], crp)

    # ---- main loop: chunk-PAIRS.  Two 4x TS fills, one DVE TT + one
    # GPSIMD TT over the flattened [128, 2048], four accumulated matmuls.
    po = [ps_o.tile([D + 1, 512], f32, tag=f"po{i}", name=f"po{i}") for i in range(2)]
    M = mybir.AluOpType.mult
    # Per pair: one vp fill (two TS), then ONE DVE TT and ONE GPSIMD TT
    # into two single-producer q tiles.  The DVE/GPSIMD column boundary
    # alternates 1536/512 and 1024/1024 to balance engine load while every
    # matmul still reads a 512-slice with a single producer.
    NP = JC // 2
    for p in range(NP):
        jc0 = 2 * p
        mt = mts[jc0 // CG]
        k = jc0 % CG
        vp = work.tile([128, 2, R], fp16, tag="vp", bufs=8, name=f"vp{p}")
        for half in range(2):
            jc = jc0 + half
            # v = max(C_rep * exp(e2[jc]), exp(.2 e2[jc]))  (= B2 * max(CD, 1))
            nc.vector.tensor_scalar(
                vp[:, half, :],
                C_rep,
                db_all[:, jc : jc + 1],
                b2_all[:, jc : jc + 1],
                M,
                mybir.AluOpType.max,
            )
        bnd = 1536 if p % 2 == 0 else 1024
        vf = vp.rearrange("p a b -> p (a b)")
        mf = mt[:, k : k + 2, :].rearrange("p a b -> p (a b)")
        qD = work.tile([128, 1536], fp16, tag="qD", bufs=8, name=f"qD{p}")
        qG = work.tile([128, 1024], fp16, tag="qG", bufs=8, name=f"qG{p}")
        nc.gpsimd.tensor_tensor(qG[:, 0 : 2048 - bnd], vf[:, bnd:], mf[:, bnd:], M)
        nc.vector.tensor_tensor(qD[:, 0:bnd], vf[:, 0:bnd], mf[:, 0:bnd], M)
        for half in range(2):
            jc = jc0 + half
            for i in range(2):
                c0 = half * 1024 + i * 512
                src_q = qD[:, c0 : c0 + 512] if c0 < bnd else qG[:, c0 - bnd : c0 - bnd + 512]
                nc.tensor.matmul(
                    po[i],
                    h_sb[:, jc, :],
                    src_q,
                    start=(jc == 0),
                    stop=(jc == JC - 1),
                )

    # ---- normalize: out = numer * (1/denom).  Broadcast the denominator
    # row via a K=1 outer product FIRST, then reciprocal on 64 lanes.
    # The two halves are interleaved to pipeline across engines.
    dcps, rrs, rsbs, ots = [], [], [], []
    for i in range(2):
        dcp = outp.tile([1, 512], f32, tag=f"dcp{i}", name=f"dcp{i}")
        nc.vector.tensor_copy(dcp, po[i][D : D + 1, :])
        dcps.append(dcp)
    for i in range(2):
        rr = ps_r.tile([D, 512], f32, tag="rr", name=f"rr{i}")
        nc.tensor.matmul(rr, ones_row_f, dcps[i], start=True, stop=True)
        rr_sb = outp.tile([D, 512], f32, tag=f"rr_sb{i}", name=f"rr_sb{i}")
        nc.vector.reciprocal(rr_sb, rr)
        rsbs.append(rr_sb)
    for i in range(2):
        o_t = outp.tile([D, 512], f32, tag=f"o_t{i}", name=f"o_t{i}")
        nc.vector.tensor_tensor(o_t, po[i][0:D, :], rsbs[i], M)
        nc.gpsimd.dma_start(out=outT[:, i * 512 : (i + 1) * 512], in_=o_t)


def build_nc():
    nc = bacc.Bacc("TRN2", num_devices=N_CORES)
    mask16_c = nc.dram_tensor("mask16_c", [N, R], fp16, kind="ExternalInput")
    x_h = nc.dram_tensor("x_h", [F_IN, N], fp16, kind="ExternalInput")
    trans_h = nc.dram_tensor("trans_h", [F_IN, D], fp16, kind="ExternalInput")
    transT_h = nc.dram_tensor("transT_h", [D, F_IN], fp16, kind="ExternalInput")
    a12h = nc.dram_tensor("a12h", [D, 2], fp16, kind="ExternalInput")
    outT = nc.dram_tensor("outT", [D, R], f32, kind="ExternalOutput")
    with ExitStack() as ctx:
        tc = ctx.enter_context(tile.TileContext(nc))
        build_kernel(
            ctx, tc, mask16_c[:, :], x_h[:, :], trans_h[:, :],
            transT_h[:, :], a12h[:, :], outT[:, :],
        )
    nc.compile()
    return nc


LAST_RESULTS = None


def kernel(x, mask, trans, attn, _trace=False):
    x16 = np.ascontiguousarray(np.asarray(x), dtype=np.float16)
    mask16 = np.ascontiguousarray(np.asarray(mask), dtype=np.float16)
    trans16 = np.ascontiguousarray(np.asarray(trans), dtype=np.float16)
    transT16 = np.ascontiguousarray(np.asarray(trans).T, dtype=np.float16)
    attn = np.asarray(attn, dtype=np.float16)
    a12 = np.ascontiguousarray(np.concatenate([attn[:D], attn[D:]], axis=1))

    nc = build_nc()
    # identical SPMD program on every core: roll x rows / mask columns by
    # -c*R so each core's own rows are always j-chunks 0..7 (a column
    # permutation inside the softmax sum; the result is unchanged)
    in_maps = [
        {
            "mask16_c": np.ascontiguousarray(
                np.roll(mask16[c * R : (c + 1) * R], -c * R, axis=1).T
            ),
            "x_h": np.ascontiguousarray(np.roll(x16, -c * R, axis=0).T),
            "trans_h": trans16,
            "transT_h": transT16,
            "a12h": a12,
        }
        for c in range(N_CORES)
    ]
    res = run_bass_kernel_spmd(nc, in_maps, list(range(N_CORES)), trace=_trace)
    global LAST_RESULTS
    LAST_RESULTS = res
    out = np.concatenate(
        [res.results[c]["outT"].T for c in range(N_CORES)], axis=0
    )
    return np.ascontiguousarray(out, dtype=np.float32)


if __name__ == "__main__":
    nc = build_nc()
    print("built OK")
